# revision 55
# baseline (speedup 1.0000x reference)
"""Malvar demosaic on 8 trn2 NeuronCores (banded bf16 matmuls, 17 bands).

Input CFA [16,1,1024,1024] f32 + four 5x5 kernels -> output [16,3,1024,1024].

Strategy (pure data parallel, 2 images per core):
  - 17 bands of up to 124 output rows per core: 8 over image 0, a BRIDGE
    band that packs image 0's last 32 rows and image 1's first 90 rows
    (separated by 2 zero partitions; its weight set zeroes the 2 junk
    output columns), then 8 over image 1.  The 128-row input band incl.
    the +-2 halo loads in natural row order with one SWDGE DMA.
  - The f32 band is converted to bf16 with a DVE copy (SBUF->SBUF 2x_2p
    mode), then two DVE shifted adds build the +-1/+-2 horizontal tap
    sums s1/s2 (bf16 runs at 2 elem/cycle).
  - The demosaic folds into banded [128]x[124,512] bf16 matmuls: per
    (channel, col-parity) a banded lhsT applies the selected conv's
    vertical taps (or the identity passthrough) per output-row parity;
    horizontal taps ride on stride-2 rhs slices of x/s1/s2.  18 matmuls
    per band; cost is output-width-bound so the PE is the bottleneck.
  - Each channel's two parity PSUM halves are evicted with one
    interleaving copy into a packed [124, 3*1024] f32 plane (PSUM reads
    are only legal from Act/DVE), then one HWDGE store DMA writes the
    band with 4KB/row descriptors (rows natural, channels via 3-level
    AP).  The last two bands store per-channel so no big DMA trails.
  - Every engine is one serial ~360GB/s lane for its DMAs + compute, so
    work is spread: PE matmuls; DVE cvt/s1/s2 + G (+ some B) evictions;
    Act R (+ most B) evictions + some stores; Pool loads + 2 late
    stores; SP most stores.  Preprocessing runs one band ahead of the
    matmuls, loads prefetch three ahead, and dummy zero matmuls bridge
    the startup so the PE p-state ramp never resets (an idle PE halves
    matmul throughput for the next 3us).
"""

import numpy as np
import ml_dtypes

import concourse.bass as bass
import concourse.mybir as mybir
import concourse.tile as tile
from concourse.bass_utils import run_bass_kernel_spmd

B, H, W = 16, 1024, 1024
N_CORES = 8
IMGS_PER_CORE = B // N_CORES
BAND = 124
NMAT = 18

# source per (channel, row-parity, col-parity): conv index 0..3 or "X"
_SEL = {
    (0, 0, 0): "X", (0, 0, 1): 1, (0, 1, 0): 2, (0, 1, 1): 3,   # R
    (1, 0, 0): 0, (1, 0, 1): "X", (1, 1, 0): "X", (1, 1, 1): 0,  # G
    (2, 0, 0): 3, (2, 0, 1): 2, (2, 1, 0): 1, (2, 1, 1): "X",    # B
}
# plane ids: 0=x, 1=s1, 2=s2 (no engine legally computes a fused
# 2*s1-s2 plane cheaply, so G also runs 3 planes)
_PLANES = {0: (0, 1, 2), 1: (0, 1, 2), 2: (0, 1, 2)}


def _mk_bands():
    """17 band descriptors: loads (p0, img, row, n), zpads (p0, n),
    stores (img, r0, p0, n_rows), wt_set."""
    bands = []
    for r0 in range(0, 992 - 123, BAND):          # image 0: r0 = 0..868
        loads = [(2 if r0 == 0 else 0, 0, max(r0 - 2, 0),
                  126 if r0 == 0 else 128)]
        # band 0's top halo is zeroed by a one-time memset, not a zpad DMA
        bands.append(dict(loads=loads, zpads=[],
                          stores=[(0, r0, 0, BAND)], wt=0))
    bands.append(dict(                             # bridge band
        loads=[(0, 0, 990, 34), (36, 1, 0, 92)], zpads=[(34, 2)],
        stores=[(0, 992, 0, 32), (1, 0, 34, 90)], wt=1))
    for r0 in range(90, 1024 - 1, BAND):           # image 1: r0 = 90..958
        n_rows = min(BAND, H - r0)
        vhi = min(r0 + 126, H)
        loads = [(0, 1, r0 - 2, vhi - (r0 - 2))]
        zpads = [] if vhi == r0 + 126 else [(vhi - (r0 - 2), 2)]
        bands.append(dict(loads=loads, zpads=zpads,
                          stores=[(1, r0, 0, n_rows)], wt=0))
    assert len(bands) == 17
    return bands


def _build_matrices(k5s):
    """Packed lhsT [128, 2*NMAT*124] bf16 (normal set + bridge set)."""
    packed = np.zeros((128, 2 * NMAT * 124), dtype=np.float32)
    idx = 0
    for ch in range(3):
        for cp in range(2):
            sels = [_SEL[(ch, rp, cp)] for rp in range(2)]
            planes = _PLANES[ch]
            mats = {pl: np.zeros((128, BAND), dtype=np.float32)
                    for pl in planes}
            for m in range(BAND):
                sel = sels[m % 2]
                if sel == "X":
                    mats[0][m + 2, m] = 1.0
                    continue
                k5 = k5s[sel]
                assert np.allclose(k5[:, 1], k5[:, 3])
                assert np.allclose(k5[:, 0], k5[:, 4])
                for q in range(-2, 3):
                    p = m + 2 + q
                    mats[0][p, m] += k5[2 + q, 2]
                    mats[1][p, m] += k5[2 + q, 1]
                    mats[2][p, m] += k5[2 + q, 0]
            for pl in planes:
                packed[:, idx * BAND:(idx + 1) * BAND] = mats[pl]
                bm = mats[pl].copy()
                bm[:, 32:34] = 0.0   # bridge set: zero the 2 seam columns
                packed[:, (NMAT + idx) * BAND:(NMAT + idx + 1) * BAND] = bm
                idx += 1
    assert idx == NMAT
    return packed.astype(ml_dtypes.bfloat16)


_CACHE = {}


def _split_waits(nc, max_waits=1):
    """The walrus in this container rejects instructions carrying more than
    one sem wait.  Hoist extra waits onto same-engine NoOps inserted right
    before the offending instruction (sequencer waits are executed in
    program order, so this is semantics-preserving)."""
    total = 0
    for bb in nc.main_func.blocks:
        insts = bb.bb.instructions if hasattr(bb, "bb") else bb.instructions
        i = 0
        while i < len(insts):
            ins = insts[i]
            si = ins.sync_info
            if si is not None and si.on_wait and len(si.on_wait) > max_waits:
                waits = list(si.on_wait)
                keep, hoist = waits[-max_waits:], waits[:-max_waits]
                nops = []
                for w in hoist:
                    nop = mybir.InstNoOp(
                        name=nc.get_next_instruction_name(),
                        engine=ins.engine, ins=[], outs=[],
                        sync_info=mybir.SyncInfo(on_wait=[w], on_update=[]))
                    nc.register_instruction(nop)
                    nops.append(nop)
                ins.sync_info = mybir.SyncInfo(
                    on_wait=keep, on_update=list(si.on_update or []))
                insts[i:i] = nops
                i += len(nops)
                total += len(nops)
            i += 1
    return total


def _build_nc():
    f32 = mybir.dt.float32
    bf16 = mybir.dt.bfloat16
    nc = bass.Bass(target_bir_lowering=False, trn_type="TRN2")
    x = nc.dram_tensor("x", [IMGS_PER_CORE, 1, H, W], f32,
                       kind="ExternalInput")
    wts = nc.dram_tensor("wm", [128, 2 * NMAT * BAND], bf16,
                         kind="ExternalInput")
    zpad = nc.dram_tensor("zpad", [2, W], f32, kind="ExternalInput")
    out = nc.dram_tensor("out", [IMGS_PER_CORE, 3, H, W], f32,
                         kind="ExternalOutput")

    bands = _mk_bands()
    NB = len(bands)
    NX = 4   # xt/xtb ring depth (loads prefetch 3 bands ahead)
    NS = 3   # s1/s2/plane ring depth
    # lane assignment for band stores: 'P' rides Pool SWDGE (issued two
    # bands late, when Pool has no loads left), 'A' the Act HWDGE queue
    # (issued one band late), 'S' the SP queue; the last band is stored
    # per-channel across Act/Pool/Act as each eviction lands
    STORE_LANE = "SSASSPSSASSASPSS"
    # a few loads ride SP to keep the Pool lane under the PE budget
    SP_LOADS = {6, 12}
    # B-channel eviction lane per band (R is always Act, G always DVE).
    # PSUM reads are only legal from Act/DVE (the BIR verifier rejects
    # GPSIMD-PSUM access); 'D' on 15 so its store can start early
    EVB_LANE = "ADDAADAADADADDADA"

    with tile.TileContext(nc) as tc:
        with (
            tc.tile_pool(name="wpool", bufs=1) as wpool,
            tc.tile_pool(name="xpool", bufs=1) as xpool,
            tc.tile_pool(name="spool", bufs=1) as spool,
            tc.tile_pool(name="opool", bufs=1) as opool,
            tc.tile_pool(name="psum", bufs=1, space="PSUM") as pspool,
        ):
            # weights split in three tiles so band 0's R matmuls only wait
            # on the small first chunk (2.8us if loaded as one DMA)
            wtA = wpool.tile([128, 6 * BAND], bf16)      # normal R mats
            wtB = wpool.tile([128, (NMAT - 6) * BAND], bf16)  # normal G+B
            wtBr = wpool.tile([128, NMAT * BAND], bf16)  # bridge set
            warm = wpool.tile([128, 8], f32)
            wrm2 = wpool.tile([128, 512], bf16)

            def wt_slice(woff, k):
                """lhsT for matrix k of weight set woff (0 normal, 1 bridge)."""
                if woff:
                    return wtBr[:, k * BAND:(k + 1) * BAND]
                if k < 6:
                    return wtA[:, k * BAND:(k + 1) * BAND]
                return wtB[:, (k - 6) * BAND:(k - 5) * BAND]

            xts = [xpool.tile([128, W + 4], f32, name=f"xt{i}")
                   for i in range(NX)]
            xtbs = [xpool.tile([128, W + 4], bf16, name=f"xtb{i}")
                    for i in range(NX)]
            s1s = [spool.tile([128, W], bf16, name=f"s1_{i}")
                   for i in range(NS)]
            s2s = [spool.tile([128, W], bf16, name=f"s2_{i}")
                   for i in range(NS)]
            planes = [opool.tile([128, 3 * W], f32, name=f"pl{i}")
                      for i in range(NS)]
            pss = [pspool.tile([BAND, 1024], f32, name=f"ps{i}")
                   for i in range(4)]

            # warm the Act table (first Activation pays a table load)
            nc.vector.memset(warm[:], 0.0)
            nc.scalar.copy(warm[:], warm[:])
            # keep the PE continuously busy on zeros until the first real
            # matmul's inputs land: the p-state ramp needs >3us of
            # uninterrupted work to reach 2.4GHz, and an idle PE resets it
            nc.vector.memset(wrm2[:], 0.0)
            for d in range(8):
                nc.tensor.matmul(pss[3][:, 0:512], wrm2[:, 0:BAND],
                                 wrm2[:, 0:512], start=True, stop=True)
            # one-time 2-col left/right halo zero per ring buffer, plus
            # band 0's 2 top halo rows (only band 0 uses xt0 unloaded there)
            for xt in xts:
                nc.vector.memset(xt[:, 0:2], 0.0)
                nc.vector.memset(xt[:, W + 2:W + 4], 0.0)
            nc.vector.memset(xts[0][0:2, 2:W + 2], 0.0)

            def issue_load(i, split=False):
                xt = xts[i % NX]
                engs = [nc.sync, nc.gpsimd] if i in SP_LOADS \
                    else [nc.gpsimd, nc.sync]   # bridge: 2nd piece on SP
                for k, (p0, b, row, n) in enumerate(bands[i]["loads"]):
                    if split:   # halve first-load latency across two lanes
                        h = n // 2
                        nc.gpsimd.dma_start(xt[p0:p0 + h, 2:W + 2],
                                            x[b, 0, row:row + h, :])
                        nc.sync.dma_start(xt[p0 + h:p0 + n, 2:W + 2],
                                          x[b, 0, row + h:row + n, :])
                    else:
                        engs[min(k, 1)].dma_start(xt[p0:p0 + n, 2:W + 2],
                                                  x[b, 0, row:row + n, :])
                for p0, n in bands[i]["zpads"]:
                    nc.gpsimd.dma_start(xt[p0:p0 + n, 2:W + 2], zpad[0:n, :])

            def prep(i):
                xt, xtb = xts[i % NX], xtbs[i % NX]
                nc.vector.tensor_copy(xtb[:], xt[:])

            def prep2(i):
                xtb = xtbs[i % NX]
                s1, s2 = s1s[i % NS], s2s[i % NS]
                nc.vector.tensor_tensor(
                    s1[:], xtb[:, 1:W + 1], xtb[:, 3:W + 3],
                    mybir.AluOpType.add)
                nc.vector.tensor_tensor(
                    s2[:], xtb[:, 0:W], xtb[:, 4:W + 4], mybir.AluOpType.add)

            def emit_store(i, *chans):
                plane = planes[i % NS]
                for k, (b, r0, p0, n_rows) in enumerate(bands[i]["stores"]):
                    chans[min(k, len(chans) - 1)].dma_start(
                        out[b, :, r0:r0 + n_rows, :].rearrange(
                            "c h w -> h c w"),
                        plane[p0:p0 + n_rows, :].rearrange(
                            "p (c w) -> p c w", c=3))

            issue_load(0, split=True)
            nc.sync.dma_start(wtA[:], wts[:, 0:6 * BAND])
            nc.sync.dma_start(wtB[:], wts[:, 6 * BAND:NMAT * BAND])
            issue_load(1)
            issue_load(2)
            prep(0)
            prep2(0)
            nps = 0
            act_store_q = []
            pool_store_q = []
            for i in range(NB):
                if i + 3 < NB:
                    issue_load(i + 3)
                if i == 0:
                    nc.scalar.dma_start(wtBr[:], wts[:, NMAT * BAND:])
                if i + 1 < NB:
                    prep(i + 1)       # cvt + s1/s2/u gate the NEXT band's
                    prep2(i + 1)      # matmuls, so they lead the DVE lane

                xtb = xtbs[i % NX]
                s1, s2 = s1s[i % NS], s2s[i % NS]
                plane = planes[i % NS]
                woff = bands[i]["wt"]
                last = i == NB - 1

                # B-channel eviction engine rotates to balance lanes; the
                # last two bands store per-channel so each store fires as
                # early as possible on an idle lane (no trailing 1.5MB DMA)
                if last:
                    ev_eng = {0: nc.scalar, 1: nc.scalar, 2: nc.vector}
                else:
                    ev_eng = {0: nc.scalar, 1: nc.vector,
                              2: {"A": nc.scalar,
                                  "D": nc.vector}[EVB_LANE[i]]}
                st_chans = None
                if last:
                    st_chans = (nc.gpsimd, nc.scalar, nc.gpsimd)
                elif i == NB - 2:
                    st_chans = (nc.sync, nc.sync, nc.sync)

                idx = 0
                for ch in range(3):
                    ps = pss[nps % 4]
                    nps += 1
                    pls = _PLANES[ch]
                    for cp in range(2):
                        for j, pl in enumerate(pls):
                            rhs = (xtb[:, 2 + cp:2 + cp + W:2] if pl == 0
                                   else (None, s1, s2)[pl][:, cp:W:2])
                            nc.tensor.matmul(
                                ps[:, cp * 512:(cp + 1) * 512],
                                wt_slice(woff, idx + j),
                                rhs,
                                start=(j == 0), stop=(j == len(pls) - 1))
                        idx += len(pls)
                    # interleave both parities back into natural cols
                    src = ps[:].rearrange("p (cp w) -> p w cp", cp=2)
                    dst = plane[0:BAND, ch * W:(ch + 1) * W].rearrange(
                        "p (w cp) -> p w cp", cp=2)
                    eng = ev_eng[ch]
                    if eng is nc.scalar:
                        eng.copy(dst, src)
                    else:
                        eng.tensor_copy(dst, src)
                    if st_chans is not None:  # store each channel as it lands
                        b, r0, p0, n_rows = bands[i]["stores"][0]
                        st_chans[ch].dma_start(
                            out[b, ch, r0:r0 + n_rows, :],
                            plane[p0:p0 + n_rows, ch * W:(ch + 1) * W])
                    if ch == 0:
                        # Act-lane stores ride one band late, right after
                        # ev_R, so their SEQ wait is ~zero and ev_B can
                        # still slip behind them (2 bands of PSUM slack)
                        while act_store_q and act_store_q[0] < i:
                            emit_store(act_store_q.pop(0),
                                       nc.scalar, nc.gpsimd)
                        while pool_store_q and pool_store_q[0] < i - 1:
                            emit_store(pool_store_q.pop(0), nc.gpsimd)

                if st_chans is None:
                    lane = STORE_LANE[i]
                    if lane == "A":
                        act_store_q.append(i)
                    elif lane == "P":
                        pool_store_q.append(i)
                    else:
                        emit_store(i, nc.sync)
            while act_store_q:
                emit_store(act_store_q.pop(0), nc.scalar, nc.gpsimd)
            while pool_store_q:
                emit_store(pool_store_q.pop(0), nc.gpsimd)

    _split_waits(nc)
    nc.finalize()
    return nc


def _get_nc():
    if "nc" not in _CACHE:
        _CACHE["nc"] = _build_nc()
    return _CACHE["nc"]


def kernel(CFA_inputs, GR_GB, Rg_RB_Bg_BR, Rg_BR_Bg_RB, Rb_BB_Br_RR,
           _trace=False):
    cfa = np.ascontiguousarray(np.asarray(CFA_inputs, dtype=np.float32))
    k5s = [np.asarray(k, dtype=np.float32)
           for k in (GR_GB, Rg_RB_Bg_BR, Rg_BR_Bg_RB, Rb_BB_Br_RR)]
    nc = _get_nc()

    wm = _build_matrices(k5s)
    zpad = np.zeros((2, W), dtype=np.float32)
    in_maps = [{"x": cfa[c * IMGS_PER_CORE:(c + 1) * IMGS_PER_CORE],
                "wm": wm, "zpad": zpad} for c in range(N_CORES)]

    res = run_bass_kernel_spmd(nc, in_maps, core_ids=list(range(N_CORES)),
                               trace=_trace)
    outs = np.concatenate([res.results[c]["out"] for c in range(N_CORES)],
                          axis=0)
    if _trace:
        kernel._last = res
    return outs
